# revision 8
# baseline (speedup 1.0000x reference)
"""CKY inside-outside (log-semiring partition function + span marginals) on 8 TRN2 cores.

Problem: nn_CFGMentionProposer — B=32 sentences, L=128 positions.
  Z[b]       = inside log-partition of sentence b (CKY, log semiring)
  marg[b,i,j]= dZ[b]/ds[b,i,j]  (span marginals via outside algorithm)

Sharding: data-parallel over sentences, 4 per NeuronCore, no collectives.

Algorithm (per core, NB=4 sentences), all charts kept in *exp space* with a
linear normalization  EA[i,d] = exp(A[i,i+d] - ALPHA*d - BETA)  chosen so all
values fit comfortably in f32 (empirical deviation of A - ALPHA*d - BETA is
within +-24 for N(0,1) scores; f32 handles +-85).

Layouts are diagonal-major [partition = span start i (or end j), free =
(sentence b, width d)].  TRN2 compute engines cannot address SBUF at partition
offsets other than 0/32/64/96, so every partition-shifted operand is
materialized by a TensorEngine matmul against a static shift matrix
SHIFT[k, c] = 1{c == k+128}: slicing SHIFT at free offset 128+w gives an
up-shift-by-w, at 128-w a down-shift-by-w (exact data movement through PSUM).

To keep the big shift matmuls OFF the critical chain, each step's shifted
operand is split into a "fresh" part (only the column produced by the
previous step, shifted by a constant 1 -> tiny matmul) and an "old" part
(everything older, already final one step earlier -> big matmul that overlaps
the previous step's DVE work).

Inside (w = 1..127), S[i,b] = sum_m EAs[i,m] * cell(i+m+1, i+w):
  psRC m-major [p, (m,b)]:
    m=0   (fresh): up-shift_1(EAs col w-1)
    m>=1  (old):   up-shift_w(EARev cols C-w+1..C-1)
  EARev col C-w = down-shift_{w-1}(EAs col w-1)   (feeds next step's "old")
  EAs col w = (sum_m EAs[:,m] * psRC) * EFs col w,  EFs = exp(softplus(s)+BETA-ALPHA)

Outside (d = 127..0), charts ECHs/ECHe hold exp(Bhat + F):
  term1[i] = sum_t ECHs[i, d+t] * up_{d+1}(EAs)[i, t-1]        (EAs static)
  term2[p] = sum_u up_{d+1}(ECHe)[p, d+u] * EARev_rev[p, u-1]
             u=1 (fresh) collapses: up_{d+1}(ECHe col d+1) == ECHs col d+1
  Stot = term1 + down_1(term2)
  ECHs[:, d] += Stot * EFs[:, d];  ECHe[:, d] = down_d(ECHs[:, d])
  (outside init pre-seeded into ECHs row 0 as M0/EAs_row0 * exp(F)_row0)

marg = sigmoid*(1-sigmoid) ⊙ EAs ⊙ ECHs;  Z from EAs row 0 via one-hot mask.
"""

import os
import numpy as np

L = 128
C = 128
NB = 4  # sentences per core
NCORES = 8
B = 32
ALPHA = 3.1153
BETA = -3.6947

_NC_CACHE = {}


def _build_nc():
    import concourse.bacc as bacc
    import concourse.mybir as mybir
    import concourse.tile as tile

    F32 = mybir.dt.float32
    use_f32r = os.environ.get("BASS_CKY_F32R", "0") == "1"
    F32R = mybir.dt.float32r if use_f32r else F32
    AF = mybir.ActivationFunctionType
    OP = mybir.AluOpType
    AX = mybir.AxisListType

    nc = bacc.Bacc(None)
    for v in (float(BETA - ALPHA), float(-BETA)):
        t = nc.alloc_sbuf_tensor(f"constx-{v}", [128, 1], F32)
        nc.gpsimd.memset(t.ap(), v)
        nc.const_aps.aps[(F32, v)] = t.ap()
    nc.all_engine_barrier()
    d_sdiag = nc.declare_dram_parameter("sdiag", [L, NB * C], F32, isOutput=False)
    d_shift = nc.declare_dram_parameter("shiftm", [L, 2 * L], F32, isOutput=False)
    d_m0z = nc.declare_dram_parameter("m0z", [1, NB * C], F32, isOutput=False)
    d_zcst = nc.declare_dram_parameter("zcst", [1, NB], F32, isOutput=False)
    d_marg = nc.declare_dram_parameter("marg", [L, NB * C], F32, isOutput=True)
    d_zout = nc.declare_dram_parameter("zout", [1, NB], F32, isOutput=True)

    def bc(t, p0=0, p1=L):
        # [p0:p1] partitions, free viewed as (b, c): [p, b, c]
        return t[p0:p1].rearrange("p (b c) -> p b c", b=NB)

    def cb(t, p0=0, p1=L):
        # [p0:p1] partitions, free viewed c-major: [p, c, b]
        return t[p0:p1].rearrange("p (b c) -> p c b", b=NB)

    def mb(t, p0, p1, m0, m1):
        # m-major flat tile viewed [p, m, b]
        return t[p0:p1].rearrange("p (m b) -> p m b", b=NB)[:, m0:m1, :]

    def bm(t, p0, p1, m0, m1):
        # m-major flat tile viewed [p, b, m] (for X-axis reductions over m)
        return t[p0:p1].rearrange("p (m b) -> p b m", b=NB)[:, :, m0:m1]

    with tile.TileContext(nc) as tc:
        with tc.tile_pool(name="sb", bufs=1) as pool:
            sdiag = pool.tile([L, NB * C], F32, tag="sdiag")
            shiftm = pool.tile([L, 2 * L], F32, tag="shiftm")
            m0z = pool.tile([1, NB * C], F32, tag="m0z")
            zcst = pool.tile([1, NB], F32, tag="zcst")
            Fs = pool.tile([L, NB * C], F32, tag="Fs")
            EFs = pool.tile([L, NB * C], F32, tag="EFs")
            SG = pool.tile([L, NB * C], F32, tag="SG")
            SPW = pool.tile([L, NB * C], F32, tag="SPW")
            EAs = pool.tile([L, NB * C], F32, tag="EAs")
            EARev = pool.tile([L, NB * C], F32, tag="EARev")
            ECHs = pool.tile([L, NB * C], F32, tag="ECHs")
            ECHe = pool.tile([L, NB * C], F32, tag="ECHe")
            MG = pool.tile([L, NB * C], F32, tag="MG")
            S1 = pool.tile([L, NB], F32, tag="S1")
            S2p = pool.tile([L, NB], F32, tag="S2p")
            zr = pool.tile([1, NB * C], F32, tag="zr")
            zs = pool.tile([1, NB], F32, tag="zs")
            rrow = pool.tile([1, NB * C], F32, tag="rrow")
            m0ef = pool.tile([1, NB * C], F32, tag="m0ef")

            shiftm_r = shiftm[:].bitcast(F32R) if use_f32r else shiftm[:]

            nc.sync.dma_start(sdiag[:], d_sdiag[:])
            nc.sync.dma_start(shiftm[:], d_shift[:])
            nc.sync.dma_start(m0z[:], d_m0z[:])
            nc.sync.dma_start(zcst[:], d_zcst[:])

            # ---- preprocessing ----
            # softplus via Ln(exp(s) + 1): Softplus isn't in CoreSim's ACT set
            nc.scalar.activation(Fs[:], sdiag[:], AF.Exp)
            nc.scalar.activation(Fs[:], Fs[:], AF.Ln, bias=1.0)
            nc.scalar.activation(EFs[:], Fs[:], AF.Exp, bias=float(BETA - ALPHA))
            nc.scalar.activation(SG[:], sdiag[:], AF.Sigmoid)
            nc.vector.tensor_tensor(out=SPW[:], in0=SG[:], in1=SG[:], op=OP.mult)
            nc.vector.tensor_tensor(out=SPW[:], in0=SG[:], in1=SPW[:], op=OP.subtract)
            nc.gpsimd.memset(EAs[:], 0.0)
            nc.gpsimd.memset(EARev[:], 0.0)
            nc.gpsimd.memset(ECHs[:], 0.0)
            nc.gpsimd.memset(ECHe[:], 0.0)
            nc.gpsimd.memset(S1[:], 0.0)
            nc.gpsimd.memset(S2p[:], 0.0)
            # EAs col 0 = exp(F_diag - BETA)
            nc.scalar.activation(
                bc(EAs)[:, :, 0:1], bc(Fs)[:, :, 0:1], AF.Exp, bias=float(-BETA)
            )

            def r(ap):
                return ap.bitcast(F32R) if use_f32r else ap

            # ---- inside pass ----
            with tc.tile_pool(name="psA", bufs=1, space="PSUM") as psA:
                for w in range(1, L):
                    psv = psA.tile([L, NB], F32, tag="psv")
                    psRC = psA.tile([L, NB * C], F32, tag="psrc")
                    prod = pool.tile([L, NB * C], F32, tag="prod")
                    S = pool.tile([L, NB], F32, tag="S")
                    # old part of RC: up-shift_w of EARev cols [C-w+1 : C]
                    if w >= 2:
                        nc.tensor.matmul(
                            out=psRC[0 : L - w, NB : NB * w],
                            lhsT=shiftm_r[:, L + w : 2 * L],
                            rhs=r(cb(EARev)[:, C - w + 1 : C, :]),
                            start=True,
                            stop=True,
                        )
                    # fresh part (m=0): up-shift_1 of EAs col w-1
                    nc.tensor.matmul(
                        out=psRC[0 : L - 1, 0:NB],
                        lhsT=shiftm[:, L + 1 : 2 * L],
                        rhs=bc(EAs)[:, :, w - 1 : w],
                        start=True,
                        stop=True,
                    )
                    # EARev col C-w = down-shift_(w-1) of EAs col w-1
                    nc.tensor.matmul(
                        out=psv[:, 0:NB],
                        lhsT=shiftm[:, L - (w - 1) : 2 * L - (w - 1)],
                        rhs=bc(EAs)[:, :, w - 1 : w],
                        start=True,
                        stop=True,
                    )
                    nc.scalar.copy(
                        out=bc(EARev)[:, :, C - w : C - w + 1],
                        in_=psv[:, 0:NB].rearrange("p (b c) -> p b c", c=1),
                    )
                    # prod[p,(m,b)] = EAs[p,(m-major)] * psRC ; S = sum_m
                    nc.vector.tensor_tensor(
                        out=mb(prod, 0, L - w, 0, w),
                        in0=cb(EAs, 0, L - w)[:, 0:w, :],
                        in1=mb(psRC, 0, L - w, 0, w),
                        op=OP.mult,
                    )
                    nc.vector.tensor_reduce(
                        out=S[0 : L - w, :],
                        in_=bm(prod, 0, L - w, 0, w),
                        axis=AX.X,
                        op=OP.add,
                    )
                    # EAs col w = S * EFs col w
                    nc.vector.tensor_tensor(
                        out=bc(EAs, 0, L - w)[:, :, w : w + 1],
                        in0=S[0 : L - w, :].rearrange("p (b c) -> p b c", c=1),
                        in1=bc(EFs, 0, L - w)[:, :, w : w + 1],
                        op=OP.mult,
                    )

            # ---- Z output ----
            nc.vector.tensor_tensor(out=zr[:], in0=m0z[:], in1=EAs[0:1, :], op=OP.mult)
            nc.vector.tensor_reduce(
                out=zs[:],
                in_=zr[:].rearrange("p (b c) -> p b c", b=NB),
                axis=AX.X,
                op=OP.add,
            )
            nc.scalar.activation(zs[:], zs[:], AF.Ln)
            nc.vector.tensor_tensor(out=zs[:], in0=zs[:], in1=zcst[:], op=OP.add)
            nc.sync.dma_start(d_zout[:], zs[:])

            # ---- outside pre-seed: ECHs row0 = m0z / EAs_row0 * exp(F)_row0 ----
            nc.vector.reciprocal(rrow[:], EAs[0:1, :])
            nc.scalar.activation(m0ef[:], Fs[0:1, :], AF.Exp)
            nc.vector.tensor_tensor(out=m0ef[:], in0=m0z[:], in1=m0ef[:], op=OP.mult)
            nc.vector.tensor_tensor(out=ECHs[0:1, :], in0=rrow[:], in1=m0ef[:], op=OP.mult)

            # ---- outside pass ----
            with tc.tile_pool(name="psB", bufs=1, space="PSUM") as psB:
                for d in range(L - 1, -1, -1):
                    T = L - 1 - d
                    psS2 = psB.tile([L, NB], F32, tag="pss2")
                    psE = psB.tile([L, NB], F32, tag="pse")
                    Stot = pool.tile([L, NB], F32, tag="Stot")
                    T1 = pool.tile([L, NB], F32, tag="T1")
                    if T > 0:
                        psRC1 = psB.tile([L, NB * C], F32, tag="psrc1")
                        psRC2 = psB.tile([L, NB * C], F32, tag="psrc2")
                        prod1 = pool.tile([L, NB * C], F32, tag="prod1")
                        prod2 = pool.tile([L, NB * C], F32, tag="prod2")
                        # RC1 = up-shift_(d+1) of EAs cols [0:T]  (static src)
                        nc.tensor.matmul(
                            out=psRC1[0 : L - 1 - d, 0 : NB * T],
                            lhsT=shiftm_r[:, L + d + 1 : 2 * L],
                            rhs=r(cb(EAs)[:, 0:T, :]),
                            start=True,
                            stop=True,
                        )
                        # term1: prod1 = ECHs cols (d+1..) * RC1 ; S1 = sum
                        nc.vector.tensor_tensor(
                            out=mb(prod1, 0, L - 1 - d, 0, T),
                            in0=cb(ECHs, 0, L - 1 - d)[:, d + 1 : L, :],
                            in1=mb(psRC1, 0, L - 1 - d, 0, T),
                            op=OP.mult,
                        )
                        nc.vector.tensor_reduce(
                            out=S1[0 : L - 1 - d, :],
                            in_=bm(prod1, 0, L - 1 - d, 0, T),
                            axis=AX.X,
                            op=OP.add,
                        )
                        # term2 fresh (u=1): up_{d+1}(ECHe col d+1) == ECHs col d+1
                        nc.vector.tensor_tensor(
                            out=mb(prod2, 0, L - 1 - d, 0, 1),
                            in0=cb(ECHs, 0, L - 1 - d)[:, d + 1 : d + 2, :],
                            in1=cb(EARev, 0, L - 1 - d)[:, C - 1 : C, :],
                            op=OP.mult,
                        )
                        # term2 old (u>=2): up-shift_(d+1) of ECHe cols [d+2:L]
                        if T >= 2:
                            nc.tensor.matmul(
                                out=psRC2[0 : L - 1 - d, NB : NB * T],
                                lhsT=shiftm_r[:, L + d + 1 : 2 * L],
                                rhs=r(cb(ECHe)[:, d + 2 : L, :]),
                                start=True,
                                stop=True,
                            )
                            nc.vector.tensor_tensor(
                                out=mb(prod2, 0, L - 1 - d, 1, T),
                                in0=mb(psRC2, 0, L - 1 - d, 1, T),
                                in1=cb(EARev, 0, L - 1 - d)[:, ::-1, :][:, 1:T, :],
                                op=OP.mult,
                            )
                        nc.vector.tensor_reduce(
                            out=S2p[0 : L - 1 - d, :],
                            in_=bm(prod2, 0, L - 1 - d, 0, T),
                            axis=AX.X,
                            op=OP.add,
                        )
                    # S2 = down-shift_1(S2p)
                    nc.tensor.matmul(
                        out=psS2[:, 0:NB],
                        lhsT=shiftm[:, L - 1 : 2 * L - 1],
                        rhs=S2p[:, 0:NB],
                        start=True,
                        stop=True,
                    )
                    # Stot = S1 + S2
                    nc.vector.tensor_tensor(
                        out=Stot[0 : L - d, :],
                        in0=S1[0 : L - d, :],
                        in1=psS2[0 : L - d, 0:NB],
                        op=OP.add,
                    )
                    # ECHs col d += Stot * EFs col d
                    nc.vector.tensor_tensor(
                        out=T1[0 : L - d, :],
                        in0=Stot[0 : L - d, :],
                        in1=bc(EFs, 0, L - d)[:, :, d : d + 1],
                        op=OP.mult,
                    )
                    nc.vector.tensor_tensor(
                        out=bc(ECHs, 0, L - d)[:, :, d : d + 1],
                        in0=bc(ECHs, 0, L - d)[:, :, d : d + 1],
                        in1=T1[0 : L - d, :].rearrange("p (b c) -> p b c", c=1),
                        op=OP.add,
                    )
                    # ECHe col d = down-shift_d(ECHs col d)
                    nc.tensor.matmul(
                        out=psE[:, 0:NB],
                        lhsT=shiftm[:, L - d : 2 * L - d],
                        rhs=bc(ECHs)[:, :, d : d + 1],
                        start=True,
                        stop=True,
                    )
                    nc.scalar.copy(
                        out=bc(ECHe)[:, :, d : d + 1],
                        in_=psE[:, 0:NB].rearrange("p (b c) -> p b c", c=1),
                    )

            # ---- marginals ----
            nc.vector.tensor_tensor(out=MG[:], in0=EAs[:], in1=ECHs[:], op=OP.mult)
            nc.vector.tensor_tensor(out=MG[:], in0=MG[:], in1=SPW[:], op=OP.mult)
            nc.sync.dma_start(d_marg[:], MG[:])
    nc.finalize()
    return nc


def get_nc():
    if "nc" not in _NC_CACHE:
        _NC_CACHE["nc"] = _build_nc()
    return _NC_CACHE["nc"]


def host_inputs(s_core, seq_core):
    """Per-core host tiles. s_core: (NB, L, L) f32 scores; seq_core: (NB,) int."""
    ii, dd = np.meshgrid(np.arange(L), np.arange(L), indexing="ij")
    jj = ii + dd
    valid = jj < L
    jc = np.minimum(jj, L - 1)
    sdiag = np.zeros((L, NB * C), np.float32)
    m0z = np.zeros((1, NB * C), np.float32)
    zcst = np.zeros((1, NB), np.float32)
    for b in range(NB):
        sdiag[:, b * C : (b + 1) * C] = np.where(valid, s_core[b][ii, jc], 0.0)
        m0z[0, b * C + int(seq_core[b]) - 1] = 1.0
        zcst[0, b] = ALPHA * (int(seq_core[b]) - 1) + BETA
    shiftm = np.zeros((L, 2 * L), np.float32)
    for k in range(L):
        shiftm[k, k + L] = 1.0
    return {"sdiag": sdiag, "shiftm": shiftm, "m0z": m0z, "zcst": zcst}


def unshard(marg_tile, z_tile):
    """marg_tile (L, NB*C) diag-major -> (NB, L, L); z_tile (1, NB) -> (NB,)"""
    ii, dd = np.meshgrid(np.arange(L), np.arange(L), indexing="ij")
    jj = ii + dd
    valid = jj < L
    out = np.zeros((NB, L, L), np.float32)
    for b in range(NB):
        M = marg_tile[:, b * C : (b + 1) * C]
        out[b][ii[valid], jj[valid]] = M[valid]
    return out, np.asarray(z_tile).reshape(NB).astype(np.float32)


def kernel(span_mention_score_matrix, sequence_lengths):
    from concourse.bass_utils import run_bass_kernel_spmd

    s = np.asarray(span_mention_score_matrix, dtype=np.float32)[..., 0]  # (B,L,L)
    seq = np.asarray(sequence_lengths).astype(np.int64)
    nc = get_nc()
    in_maps = []
    for k in range(NCORES):
        sl = slice(k * NB, (k + 1) * NB)
        in_maps.append(host_inputs(s[sl], seq[k * NB : (k + 1) * NB]))
    res = run_bass_kernel_spmd(nc, in_maps, list(range(NCORES)))
    Z = np.zeros((B,), np.float32)
    marg = np.zeros((B, L, L), np.float32)
    for k in range(NCORES):
        m_k, z_k = unshard(res.results[k]["marg"], res.results[k]["zout"])
        marg[k * NB : (k + 1) * NB] = m_k
        Z[k * NB : (k + 1) * NB] = z_k
    return Z, marg
